# revision 8
# baseline (speedup 1.0000x reference)
"""Bahdanau attention Trainium2 kernel.

Problem (per full input):
    query [32, 1024], values [32, 2048, 1024], W1 [1024, 1024],
    W2 [1024, 1024], V [1024, 1]
    q_proj = query @ W1                       # [B, U]
    v_proj = values @ W2                      # [B, T, U]
    score  = tanh(q_proj[:, None] + v_proj)   # [B, T, U]
    logits = score @ V                        # [B, T, 1]
    attn   = softmax(logits, axis=1)          # [B, T, 1]
    ctx    = sum_t attn * values              # [B, D]
    returns (ctx, attn)

Sharding: data-parallel over batch, 4 batches per core on 8 cores.

Per-core plan (u-major "transposed" layout so that the q_proj add is a
free per-partition ACT bias, the logits reduction is a PE matmul, and
the softmax runs along the free axis):
    - weights cast to bf16 on load (SWDGE cast DMA)
    - values: fp32 DRAM -> bf16 DRAM (cast DMA), then
        * xbar DMA transpose  -> valT [128(d), 8, 2048(t)]  (rhs of main mm)
        * straight load       -> vnat [128(t), 16, 1024(d)] (context mm)
    - main mm: vprojT[u, t] += W2bf[d, u].T @ valT[d, t] (8 k-chunks)
    - ACT: scoreT = tanh(vprojT + qprojT[u] bias) -> bf16
    - logits mm: lhsT = Vbf [u, 1] -> logits [1, t]
    - softmax on [1, 2048] row (DVE/ACT, exp accum_out gives sum)
    - context mm: lhsT = attnT [t, 1], rhs = vnat -> ctx [1, d]
"""
import numpy as np
from contextlib import ExitStack

import concourse.bass as bass
import concourse.bacc as bacc
import concourse.tile as tile
from concourse import mybir, masks
from concourse import bass_utils

B, T, D, U = 32, 2048, 1024, 1024
NCORES = 8
BSH = B // NCORES          # 4 batches per core
P = 128
DC = D // P                # 8 contraction chunks
UC = U // P                # 8 u tiles
NF = 512                   # matmul free dim (one PSUM bank of fp32)
TC = T // NF               # 4 t chunks
TT = T // P                # 16 t tiles
f32 = mybir.dt.float32
bf16 = mybir.dt.bfloat16
AF = mybir.ActivationFunctionType


def _body(ctx, tc, values, query, W1, W2, V, ctx_out, attn_out):
    nc = tc.nc
    const = ctx.enter_context(tc.tile_pool(name="const", bufs=1))
    dram = ctx.enter_context(tc.tile_pool(name="dram", bufs=6, space="DRAM"))
    vpool = ctx.enter_context(tc.tile_pool(name="vpool", bufs=6))
    npool = ctx.enter_context(tc.tile_pool(name="npool", bufs=1))
    spool = ctx.enter_context(tc.tile_pool(name="spool", bufs=2))
    smax = ctx.enter_context(tc.tile_pool(name="smax", bufs=2))
    mm_ps = ctx.enter_context(tc.tile_pool(name="mm_ps", bufs=3, space="PSUM"))
    lg_ps = ctx.enter_context(tc.tile_pool(name="lg_ps", bufs=2, space="PSUM"))
    cx_ps = ctx.enter_context(tc.tile_pool(name="cx_ps", bufs=1, space="PSUM"))
    tp_ps = ctx.enter_context(tc.tile_pool(name="tp_ps", bufs=1, space="PSUM"))

    # ---- constants -------------------------------------------------------
    ident = const.tile([P, P], bf16)
    masks.make_identity(nc, ident)

    W2bf = const.tile([P, DC, U], bf16)
    nc.gpsimd.dma_start(W2bf, W2.rearrange("(dc p) u -> p dc u", p=P))
    W1bf = const.tile([P, DC, U], bf16)
    nc.gpsimd.dma_start(W1bf, W1.rearrange("(dc p) u -> p dc u", p=P))
    Vbf = const.tile([P, UC, 1], bf16)
    nc.gpsimd.dma_start(Vbf, V.rearrange("(uc p) o -> p uc o", p=P))

    # ---- qprojT[u, b] = (query @ W1).T ----------------------------------
    q_sb = const.tile([BSH, D], f32)
    nc.sync.dma_start(q_sb, query)
    q_bf = const.tile([BSH, D], bf16)
    nc.vector.tensor_copy(q_bf, q_sb)
    qT = const.tile([P, DC, BSH], bf16)
    for dc in range(DC):
        tp = tp_ps.tile([P, BSH], bf16, tag="tp")
        nc.tensor.transpose(tp, q_bf[:, dc * P:(dc + 1) * P], ident[:BSH, :BSH])
        nc.vector.tensor_copy(qT[:, dc, :], tp)
    qpT = const.tile([P, UC, BSH], f32)
    for uc in range(UC):
        qp = tp_ps.tile([P, BSH], f32, tag="tp")
        for dc in range(DC):
            nc.tensor.matmul(qp, W1bf[:, dc, uc * P:(uc + 1) * P], qT[:, dc, :],
                             start=dc == 0, stop=dc == DC - 1)
        nc.vector.tensor_copy(qpT[:, uc, :], qp)

    # ---- per-batch pipeline ---------------------------------------------
    TPC = TT // TC  # t-tiles per t-chunk (4)
    for b in range(BSH):
        vnat = npool.tile([P, TT, D], bf16, tag="vnat")
        logits = smax.tile([1, T], f32, tag="logits")
        for tc4 in range(TC):
            tsl = slice(tc4 * NF, (tc4 + 1) * NF)
            # stage this t-chunk: cast fp32->bf16 into DRAM, then
            # xbar-transpose into [d, t] layout + straight copy for context
            valbf = dram.tile([NF, D], bf16, tag="valbf")
            nc.gpsimd.dma_start(valbf, values[b, tsl, :])
            valT = vpool.tile([P, DC, NF], bf16, tag="valT")
            nc.sync.dma_start_transpose(valT, valbf)
            nc.sync.dma_start(vnat[:, tc4 * TPC:(tc4 + 1) * TPC, :],
                              valbf.rearrange("(tt p) d -> p tt d", p=P))

            scoreT = spool.tile([P, UC, NF], bf16, tag="score")
            for uc in range(UC):
                ps = mm_ps.tile([P, NF], f32, tag="mm")
                for dc in range(DC):
                    nc.tensor.matmul(ps,
                                     W2bf[:, dc, uc * P:(uc + 1) * P],
                                     valT[:, dc, :],
                                     start=dc == 0, stop=dc == DC - 1)
                nc.scalar.activation(scoreT[:, uc, :], ps, AF.Tanh,
                                     bias=qpT[:, uc, b:b + 1])
            # logits: 4 concurrent col-group M=1 matmuls (partials on
            # partitions 0/32/64/96), 2 accumulation rounds, DVE combine
            lps = lg_ps.tile([P, NF], f32, tag="lg")
            for uc in range(UC):
                j = uc % 4
                nc.tensor.matmul(lps[32 * j:32 * j + 1, :],
                                 Vbf[:, uc, :], scoreT[:, uc, :],
                                 start=uc < 4, stop=uc >= 4,
                                 tile_position=(0, 32 * j))
            lgacc = smax.tile([1, NF], f32, tag="lgacc")
            nc.vector.tensor_copy(lgacc, lps[0:1, :])
            nc.vector.tensor_add(lgacc, lgacc, lps[32:33, :])
            nc.vector.tensor_add(lgacc, lgacc, lps[64:65, :])
            nc.vector.tensor_add(logits[:, tsl], lgacc, lps[96:97, :])

        # softmax over the free axis on one partition
        mx = smax.tile([1, 1], f32, tag="mx")
        nc.vector.reduce_max(mx, logits, axis=mybir.AxisListType.X)
        negmx = smax.tile([1, 1], f32, tag="negmx")
        nc.vector.tensor_scalar_mul(negmx, mx, -1.0)
        probs = smax.tile([1, T], f32, tag="probs")
        sumexp = smax.tile([1, 1], f32, tag="sumexp")
        nc.scalar.activation(probs, logits, AF.Exp, bias=negmx,
                             accum_out=sumexp)
        rse = smax.tile([1, 1], f32, tag="rse")
        nc.vector.reciprocal(rse, sumexp)
        nc.vector.tensor_scalar_mul(probs, probs, rse)
        nc.sync.dma_start(attn_out[b], probs)

        # context: ctx[1, d] = sum_t attn[t] * values[t, d]
        attn16 = smax.tile([1, T], bf16, tag="attn16")
        nc.vector.tensor_copy(attn16, probs)
        # reshape [1, 2048] -> [16, 128] via SBUF->SBUF DMA, then one
        # PE transpose -> attnT [128(t%128), 16(t/128)]
        attnR = smax.tile([TT, P], bf16, tag="attnR")
        nc.sync.dma_start(attnR, attn16)
        tpA = tp_ps.tile([P, TT], bf16, tag="tp")
        nc.tensor.transpose(tpA, attnR, ident[:TT, :TT])
        attnT = smax.tile([P, TT], bf16, tag="attnT")
        nc.vector.tensor_copy(attnT, tpA)
        cxa = cx_ps.tile([P, NF], f32, tag="cx0")
        cxb = cx_ps.tile([P, NF], f32, tag="cx1")
        for tt in range(TT):
            j = tt % 4
            nc.tensor.matmul(cxa[32 * j:32 * j + 1, :],
                             attnT[:, tt:tt + 1], vnat[:, tt, 0:NF],
                             start=tt < 4, stop=tt >= TT - 4,
                             tile_position=(0, 32 * j))
            nc.tensor.matmul(cxb[32 * j:32 * j + 1, :],
                             attnT[:, tt:tt + 1], vnat[:, tt, NF:D],
                             start=tt < 4, stop=tt >= TT - 4,
                             tile_position=(0, 32 * j))
        ctx_sb = smax.tile([1, D], f32, tag="ctx_sb")
        for cx, half in ((cxa, slice(0, NF)), (cxb, slice(NF, D))):
            hacc = smax.tile([1, NF], f32, tag="hacc")
            nc.vector.tensor_copy(hacc, cx[0:1, :])
            nc.vector.tensor_add(hacc, hacc, cx[32:33, :])
            nc.vector.tensor_add(hacc, hacc, cx[64:65, :])
            nc.vector.tensor_add(ctx_sb[:, half], hacc, cx[96:97, :])
        nc.sync.dma_start(ctx_out[b], ctx_sb)


def build():
    nc = bacc.Bacc("TRN2", target_bir_lowering=False, debug=False,
                   num_devices=NCORES)
    values = nc.dram_tensor("values", (BSH, T, D), f32,
                            kind="ExternalInput").ap()
    query = nc.dram_tensor("query", (BSH, D), f32, kind="ExternalInput").ap()
    W1 = nc.dram_tensor("W1", (D, U), f32, kind="ExternalInput").ap()
    W2 = nc.dram_tensor("W2", (D, U), f32, kind="ExternalInput").ap()
    V = nc.dram_tensor("V", (U, 1), f32, kind="ExternalInput").ap()
    ctx_out = nc.dram_tensor("ctx", (BSH, D), f32, kind="ExternalOutput").ap()
    attn_out = nc.dram_tensor("attn", (BSH, T), f32,
                              kind="ExternalOutput").ap()
    with tile.TileContext(nc) as tc, ExitStack() as ctx:
        _body(ctx, tc, values, query, W1, W2, V, ctx_out, attn_out)
    nc.compile()
    return nc


_NC = None


def _get_nc():
    global _NC
    if _NC is None:
        _NC = build()
    return _NC


def _run(in_maps, **kwargs):
    nc = _get_nc()
    return bass_utils.run_bass_kernel_spmd(nc, in_maps,
                                           core_ids=list(range(NCORES)),
                                           **kwargs)


def make_in_maps(query, values, W1, W2, V):
    query = np.ascontiguousarray(np.asarray(query, dtype=np.float32))
    values = np.ascontiguousarray(np.asarray(values, dtype=np.float32))
    W1 = np.ascontiguousarray(np.asarray(W1, dtype=np.float32))
    W2 = np.ascontiguousarray(np.asarray(W2, dtype=np.float32))
    V = np.ascontiguousarray(np.asarray(V, dtype=np.float32))
    in_maps = []
    for c in range(NCORES):
        sl = slice(c * BSH, (c + 1) * BSH)
        in_maps.append({
            "values": np.ascontiguousarray(values[sl]),
            "query": np.ascontiguousarray(query[sl]),
            "W1": W1, "W2": W2, "V": V,
        })
    return in_maps


def assemble(results):
    context = np.concatenate([results[c]["ctx"] for c in range(NCORES)], 0)
    attn = np.concatenate([results[c]["attn"] for c in range(NCORES)], 0)
    return context.astype(np.float32), attn[..., None].astype(np.float32)


def kernel(query, values, W1, W2, V):
    in_maps = make_in_maps(query, values, W1, W2, V)
    res = _run(in_maps)
    return assemble(res.results)


if __name__ == "__main__":
    rng = np.random.default_rng(0)
    scale = 1.0 / np.sqrt(D)
    inputs = dict(
        query=rng.standard_normal((B, D), dtype=np.float32),
        values=rng.standard_normal((B, T, D), dtype=np.float32),
        W1=rng.standard_normal((D, U), dtype=np.float32) * scale,
        W2=rng.standard_normal((D, U), dtype=np.float32) * scale,
        V=rng.standard_normal((U, 1), dtype=np.float32) / np.sqrt(U),
    )
    ctx_np, attn_np = kernel(**inputs)
    print(ctx_np.shape, attn_np.shape, ctx_np.dtype, attn_np.dtype)


# revision 20
# speedup vs baseline: 237.5186x; 237.5186x over previous
"""Bahdanau attention Trainium2 kernel.

Problem (per full input):
    query [32, 1024], values [32, 2048, 1024], W1 [1024, 1024],
    W2 [1024, 1024], V [1024, 1]
    q_proj = query @ W1                       # [B, U]
    v_proj = values @ W2                      # [B, T, U]
    score  = tanh(q_proj[:, None] + v_proj)   # [B, T, U]
    logits = score @ V                        # [B, T, 1]
    attn   = softmax(logits, axis=1)          # [B, T, 1]
    ctx    = sum_t attn * values              # [B, D]
    returns (ctx, attn)

Sharding: data-parallel over batch, 4 batches per core on 8 cores.

Per-core plan (u-major "transposed" layout so that the q_proj add is a
free per-partition ACT bias, the logits reduction is a PE matmul, and
the softmax runs along the free axis):
    - weights cast to bf16 on load (SWDGE cast DMA)
    - values: fp32 DRAM -> bf16 DRAM (cast DMA), then
        * xbar DMA transpose  -> valT [128(d), 8, 2048(t)]  (rhs of main mm)
        * straight load       -> vnat [128(t), 16, 1024(d)] (context mm)
    - main mm: vprojT[u, t] += W2bf[d, u].T @ valT[d, t] (8 k-chunks)
    - ACT: scoreT = tanh(vprojT + qprojT[u] bias) -> bf16
    - logits mm: lhsT = Vbf [u, 1] -> logits [1, t]
    - softmax on [1, 2048] row (DVE/ACT, exp accum_out gives sum)
    - context mm: lhsT = attnT [t, 1], rhs = vnat -> ctx [1, d]
"""
import numpy as np
from contextlib import ExitStack

import concourse.bass as bass
import concourse.bacc as bacc
import concourse.tile as tile
from concourse import mybir, masks
from concourse import bass_utils

B, T, D, U = 32, 2048, 1024, 1024
NCORES = 8
BSH = B // NCORES          # 4 batches per core
P = 128
DC = D // P                # 8 contraction chunks
UC = U // P                # 8 u tiles
NF = 512                   # matmul free dim (one PSUM bank of fp32)
TC = T // NF               # 4 t chunks
TT = T // P                # 16 t tiles
f32 = mybir.dt.float32
bf16 = mybir.dt.bfloat16
AF = mybir.ActivationFunctionType


ALL_PARTS = frozenset({"dma", "compute", "coltile", "sbuftr"})


def _body(ctx, tc, values, query, W1, W2, V, ctx_out, attn_out, loop_n=None,
          parts=ALL_PARTS):
    nc = tc.nc
    do_dma = "dma" in parts
    do_compute = "compute" in parts
    coltile = "coltile" in parts
    sbuftr = "sbuftr" in parts
    const = ctx.enter_context(tc.tile_pool(name="const", bufs=1))
    dram = ctx.enter_context(tc.tile_pool(name="dram", bufs=6, space="DRAM"))
    vpool = ctx.enter_context(tc.tile_pool(name="vpool", bufs=6))
    npool = ctx.enter_context(tc.tile_pool(name="npool", bufs=1))
    spool = ctx.enter_context(tc.tile_pool(name="spool", bufs=2))
    smax = ctx.enter_context(tc.tile_pool(name="smax", bufs=2))
    mm_ps = ctx.enter_context(tc.tile_pool(name="mm_ps", bufs=3, space="PSUM"))
    lg_ps = ctx.enter_context(tc.tile_pool(name="lg_ps", bufs=2, space="PSUM"))
    cx_ps = ctx.enter_context(tc.tile_pool(name="cx_ps", bufs=1, space="PSUM"))
    tp_ps = ctx.enter_context(tc.tile_pool(name="tp_ps", bufs=1, space="PSUM"))

    # ---- constants -------------------------------------------------------
    ident = const.tile([P, P], bf16)
    masks.make_identity(nc, ident)

    W2bf = const.tile([P, DC, U], bf16)
    nc.gpsimd.dma_start(W2bf, W2.rearrange("(dc p) u -> p dc u", p=P))
    W1bf = const.tile([P, DC, U], bf16)
    nc.gpsimd.dma_start(W1bf, W1.rearrange("(dc p) u -> p dc u", p=P))
    Vbf = const.tile([P, UC, 1], bf16)
    nc.gpsimd.dma_start(Vbf, V.rearrange("(uc p) o -> p uc o", p=P))

    # ---- qprojT[u, b] = (query @ W1).T ----------------------------------
    q_sb = const.tile([BSH, D], f32)
    nc.sync.dma_start(q_sb, query)
    q_bf = const.tile([BSH, D], bf16)
    nc.vector.tensor_copy(q_bf, q_sb)
    qT = const.tile([P, DC, BSH], bf16)
    for dc in range(DC):
        tp = tp_ps.tile([P, BSH], bf16, tag="tp")
        nc.tensor.transpose(tp, q_bf[:, dc * P:(dc + 1) * P], ident[:BSH, :BSH])
        nc.vector.tensor_copy(qT[:, dc, :], tp)
    qpT = const.tile([P, UC, BSH], f32)
    for uc in range(UC):
        qp = tp_ps.tile([P, BSH], f32, tag="tp")
        for dc in range(DC):
            nc.tensor.matmul(qp, W1bf[:, dc, uc * P:(uc + 1) * P], qT[:, dc, :],
                             start=dc == 0, stop=dc == DC - 1)
        nc.vector.tensor_copy(qpT[:, uc, :], qp)

    # ---- per-batch pipeline ---------------------------------------------
    TPC = TT // TC  # t-tiles per t-chunk (4)

    def _batches():
      for b in range(BSH):
        vnat = npool.tile([P, TT, D], bf16, tag="vnat")
        logits = smax.tile([1, T], f32, tag="logits")
        for tc4 in range(TC):
            tsl = slice(tc4 * NF, (tc4 + 1) * NF)
            # stage this t-chunk: cast fp32->bf16 into DRAM, then
            # xbar-transpose into [d, t] layout + straight copy for context
            valT = vpool.tile([P, DC, NF], bf16, tag="valT")
            if do_dma and sbuftr:
                # cast-load straight into vnat (fp32->bf16, SWDGE), then
                # SBUF->SBUF xbar transposes in [128,128] blocks (dest must
                # be free-contiguous per block for the xbar path)
                nc.gpsimd.dma_start(
                    vnat[:, tc4 * TPC:(tc4 + 1) * TPC, :],
                    values[b, tsl, :].rearrange("(tt p) d -> p tt d", p=P))
                for tt in range(TPC):
                    for dc in range(DC):
                        nc.scalar.dma_start(
                            valT[:, dc, tt * P:(tt + 1) * P],
                            vnat[:, tc4 * TPC + tt, dc * P:(dc + 1) * P],
                            transpose=True)
            elif do_dma:
                valbf = dram.tile([NF, D], bf16, tag="valbf")
                nc.gpsimd.dma_start(valbf, values[b, tsl, :])
                nc.sync.dma_start_transpose(valT, valbf)
                nc.sync.dma_start(vnat[:, tc4 * TPC:(tc4 + 1) * TPC, :],
                                  valbf.rearrange("(tt p) d -> p tt d", p=P))
            if not do_compute:
                junk = smax.tile([P, 8], bf16, tag="junk")
                nc.vector.tensor_copy(junk, valT[:, :, 0])
                nc.vector.tensor_copy(junk[:, 0:4],
                                      vnat[:, tc4 * TPC:(tc4 + 1) * TPC, 0])
                nc.vector.tensor_copy(logits[:, tc4:tc4 + 1], junk[0:1, 0:1])
                continue

            scoreT = spool.tile([P, UC, NF], bf16, tag="score")
            for uc in range(UC):
                ps = mm_ps.tile([P, NF], f32, tag="mm")
                for dc in range(DC):
                    nc.tensor.matmul(ps,
                                     W2bf[:, dc, uc * P:(uc + 1) * P],
                                     valT[:, dc, :],
                                     start=dc == 0, stop=dc == DC - 1)
                nc.scalar.activation(scoreT[:, uc, :], ps, AF.Tanh,
                                     bias=qpT[:, uc, b:b + 1])
            # logits: 4 concurrent col-group M=1 matmuls (partials on
            # partitions 0/32/64/96), 2 accumulation rounds, DVE combine
            lps = lg_ps.tile([P, NF], f32, tag="lg")
            if coltile:
                for uc in range(UC):
                    j = uc % 4
                    nc.tensor.matmul(lps[32 * j:32 * j + 1, :],
                                     Vbf[:, uc, :], scoreT[:, uc, :],
                                     start=uc < 4, stop=uc >= 4,
                                     tile_position=(0, 32 * j))
                lgacc = smax.tile([1, NF], f32, tag="lgacc")
                nc.vector.tensor_copy(lgacc, lps[0:1, :])
                nc.vector.tensor_add(lgacc, lgacc, lps[32:33, :])
                nc.vector.tensor_add(lgacc, lgacc, lps[64:65, :])
                nc.vector.tensor_add(logits[:, tsl], lgacc, lps[96:97, :])
            else:
                for uc in range(UC):
                    nc.tensor.matmul(lps[0:1, :], Vbf[:, uc, :],
                                     scoreT[:, uc, :],
                                     start=uc == 0, stop=uc == UC - 1)
                nc.vector.tensor_copy(logits[:, tsl], lps[0:1, :])

        if not do_compute:
            nc.sync.dma_start(attn_out[b], logits)
            junk2 = smax.tile([1, D], f32, tag="junk2")
            nc.vector.tensor_copy(junk2[:, 0:4], logits[:, 0:4])
            nc.sync.dma_start(ctx_out[b], junk2)
            continue

        # softmax over the free axis on one partition
        mx = smax.tile([1, 1], f32, tag="mx")
        nc.vector.reduce_max(mx, logits, axis=mybir.AxisListType.X)
        negmx = smax.tile([1, 1], f32, tag="negmx")
        nc.vector.tensor_scalar_mul(negmx, mx, -1.0)
        probs = smax.tile([1, T], f32, tag="probs")
        sumexp = smax.tile([1, 1], f32, tag="sumexp")
        nc.scalar.activation(probs, logits, AF.Exp, bias=negmx,
                             accum_out=sumexp)
        rse = smax.tile([1, 1], f32, tag="rse")
        nc.vector.reciprocal(rse, sumexp)
        nc.vector.tensor_scalar_mul(probs, probs, rse)
        nc.sync.dma_start(attn_out[b], probs)

        # context: ctx[1, d] = sum_t attn[t] * values[t, d]
        attn16 = smax.tile([1, T], bf16, tag="attn16")
        nc.vector.tensor_copy(attn16, probs)
        # reshape [1, 2048] -> [16, 128] via SBUF->SBUF DMA, then one
        # PE transpose -> attnT [128(t%128), 16(t/128)]
        attnR = smax.tile([TT, P], bf16, tag="attnR")
        nc.sync.dma_start(attnR, attn16)
        tpA = tp_ps.tile([P, TT], bf16, tag="tp")
        nc.tensor.transpose(tpA, attnR, ident[:TT, :TT])
        attnT = smax.tile([P, TT], bf16, tag="attnT")
        nc.vector.tensor_copy(attnT, tpA)
        cxa = cx_ps.tile([P, NF], f32, tag="cx0")
        cxb = cx_ps.tile([P, NF], f32, tag="cx1")
        ctx_sb = smax.tile([1, D], f32, tag="ctx_sb")
        if coltile:
            for tt in range(TT):
                j = tt % 4
                nc.tensor.matmul(cxa[32 * j:32 * j + 1, :],
                                 attnT[:, tt:tt + 1], vnat[:, tt, 0:NF],
                                 start=tt < 4, stop=tt >= TT - 4,
                                 tile_position=(0, 32 * j))
                nc.tensor.matmul(cxb[32 * j:32 * j + 1, :],
                                 attnT[:, tt:tt + 1], vnat[:, tt, NF:D],
                                 start=tt < 4, stop=tt >= TT - 4,
                                 tile_position=(0, 32 * j))
            for cx, half in ((cxa, slice(0, NF)), (cxb, slice(NF, D))):
                hacc = smax.tile([1, NF], f32, tag="hacc")
                nc.vector.tensor_copy(hacc, cx[0:1, :])
                nc.vector.tensor_add(hacc, hacc, cx[32:33, :])
                nc.vector.tensor_add(hacc, hacc, cx[64:65, :])
                nc.vector.tensor_add(ctx_sb[:, half], hacc, cx[96:97, :])
        else:
            for tt in range(TT):
                nc.tensor.matmul(cxa[0:1, :], attnT[:, tt:tt + 1],
                                 vnat[:, tt, 0:NF],
                                 start=tt == 0, stop=tt == TT - 1)
                nc.tensor.matmul(cxb[0:1, :], attnT[:, tt:tt + 1],
                                 vnat[:, tt, NF:D],
                                 start=tt == 0, stop=tt == TT - 1)
            nc.vector.tensor_copy(ctx_sb[:, 0:NF], cxa[0:1, :])
            nc.vector.tensor_copy(ctx_sb[:, NF:D], cxb[0:1, :])
        nc.sync.dma_start(ctx_out[b], ctx_sb)

    if loop_n is not None and loop_n > 1:
        with tc.For_i(0, loop_n, 1) as _i:
            _batches()
    else:
        _batches()


def build():
    nc = bacc.Bacc("TRN2", target_bir_lowering=False, debug=False,
                   num_devices=NCORES)
    values = nc.dram_tensor("values", (BSH, T, D), f32,
                            kind="ExternalInput").ap()
    query = nc.dram_tensor("query", (BSH, D), f32, kind="ExternalInput").ap()
    W1 = nc.dram_tensor("W1", (D, U), f32, kind="ExternalInput").ap()
    W2 = nc.dram_tensor("W2", (D, U), f32, kind="ExternalInput").ap()
    V = nc.dram_tensor("V", (U, 1), f32, kind="ExternalInput").ap()
    ctx_out = nc.dram_tensor("ctx", (BSH, D), f32, kind="ExternalOutput").ap()
    attn_out = nc.dram_tensor("attn", (BSH, T), f32,
                              kind="ExternalOutput").ap()
    with tile.TileContext(nc) as tc, ExitStack() as ctx:
        _body(ctx, tc, values, query, W1, W2, V, ctx_out, attn_out)
    nc.compile()
    return nc


_NC = None


def _get_nc():
    global _NC
    if _NC is None:
        _NC = build()
    return _NC


def _run(in_maps, **kwargs):
    nc = _get_nc()
    return bass_utils.run_bass_kernel_spmd(nc, in_maps,
                                           core_ids=list(range(NCORES)),
                                           **kwargs)


def make_in_maps(query, values, W1, W2, V):
    query = np.ascontiguousarray(np.asarray(query, dtype=np.float32))
    values = np.ascontiguousarray(np.asarray(values, dtype=np.float32))
    W1 = np.ascontiguousarray(np.asarray(W1, dtype=np.float32))
    W2 = np.ascontiguousarray(np.asarray(W2, dtype=np.float32))
    V = np.ascontiguousarray(np.asarray(V, dtype=np.float32))
    in_maps = []
    for c in range(NCORES):
        sl = slice(c * BSH, (c + 1) * BSH)
        in_maps.append({
            "values": np.ascontiguousarray(values[sl]),
            "query": np.ascontiguousarray(query[sl]),
            "W1": W1, "W2": W2, "V": V,
        })
    return in_maps


def assemble(results):
    context = np.concatenate([results[c]["ctx"] for c in range(NCORES)], 0)
    attn = np.concatenate([results[c]["attn"] for c in range(NCORES)], 0)
    return context.astype(np.float32), attn[..., None].astype(np.float32)


def kernel(query, values, W1, W2, V):
    in_maps = make_in_maps(query, values, W1, W2, V)
    res = _run(in_maps)
    return assemble(res.results)


if __name__ == "__main__":
    rng = np.random.default_rng(0)
    scale = 1.0 / np.sqrt(D)
    inputs = dict(
        query=rng.standard_normal((B, D), dtype=np.float32),
        values=rng.standard_normal((B, T, D), dtype=np.float32),
        W1=rng.standard_normal((D, U), dtype=np.float32) * scale,
        W2=rng.standard_normal((D, U), dtype=np.float32) * scale,
        V=rng.standard_normal((U, 1), dtype=np.float32) / np.sqrt(U),
    )
    ctx_np, attn_np = kernel(**inputs)
    print(ctx_np.shape, attn_np.shape, ctx_np.dtype, attn_np.dtype)


# revision 26
# speedup vs baseline: 463.3792x; 1.9509x over previous
"""Bahdanau attention Trainium2 kernel.

Problem (per full input):
    query [32, 1024], values [32, 2048, 1024], W1 [1024, 1024],
    W2 [1024, 1024], V [1024, 1]
    q_proj = query @ W1                       # [B, U]
    v_proj = values @ W2                      # [B, T, U]
    score  = tanh(q_proj[:, None] + v_proj)   # [B, T, U]
    logits = score @ V                        # [B, T, 1]
    attn   = softmax(logits, axis=1)          # [B, T, 1]
    ctx    = sum_t attn * values              # [B, D]
    returns (ctx, attn)

Sharding: data-parallel over batch, 4 batches per core on 8 cores.

Per-core plan (u-major "transposed" layout so that the q_proj add is a
free per-partition ACT bias, the logits reduction is a PE matmul, and
the softmax runs along the free axis):
    - weights cast to bf16 on load (SWDGE cast DMA)
    - values: fp32 DRAM -> bf16 DRAM (cast DMA), then
        * xbar DMA transpose  -> valT [128(d), 8, 2048(t)]  (rhs of main mm)
        * straight load       -> vnat [128(t), 16, 1024(d)] (context mm)
    - main mm: vprojT[u, t] += W2bf[d, u].T @ valT[d, t] (8 k-chunks)
    - ACT: scoreT = tanh(vprojT + qprojT[u] bias) -> bf16
    - logits mm: lhsT = Vbf [u, 1] -> logits [1, t]
    - softmax on [1, 2048] row (DVE/ACT, exp accum_out gives sum)
    - context mm: lhsT = attnT [t, 1], rhs = vnat -> ctx [1, d]
"""
import numpy as np
from contextlib import ExitStack

import concourse.bass as bass
import concourse.bacc as bacc
import concourse.tile as tile
from concourse import mybir, masks
from concourse import bass_utils

B, T, D, U = 32, 2048, 1024, 1024
NCORES = 8
BSH = B // NCORES          # 4 batches per core
P = 128
DC = D // P                # 8 contraction chunks
UC = U // P                # 8 u tiles
NF = 512                   # matmul free dim (one PSUM bank of fp32)
TC = T // NF               # 4 t chunks
TT = T // P                # 16 t tiles
f32 = mybir.dt.float32
bf16 = mybir.dt.bfloat16
AF = mybir.ActivationFunctionType


ALL_PARTS = frozenset({"dma", "compute", "coltile", "sbuftr2"})


def _body(ctx, tc, values, query, W1, W2, V, ctx_out, attn_out, loop_n=None,
          parts=ALL_PARTS):
    nc = tc.nc
    do_dma = "dma" in parts
    do_compute = "compute" in parts
    coltile = "coltile" in parts
    sbuftr = "sbuftr" in parts
    sbuftr2 = "sbuftr2" in parts
    const = ctx.enter_context(tc.tile_pool(name="const", bufs=1))
    dram = ctx.enter_context(tc.tile_pool(name="dram", bufs=6, space="DRAM"))
    vpool = ctx.enter_context(tc.tile_pool(name="vpool", bufs=4))
    npool = ctx.enter_context(tc.tile_pool(name="npool", bufs=2))
    spool = ctx.enter_context(tc.tile_pool(name="spool", bufs=2))
    smax = ctx.enter_context(tc.tile_pool(name="smax", bufs=2))
    mm_ps = ctx.enter_context(tc.tile_pool(name="mm_ps", bufs=3, space="PSUM"))
    lg_ps = ctx.enter_context(tc.tile_pool(name="lg_ps", bufs=2, space="PSUM"))
    cx_ps = ctx.enter_context(tc.tile_pool(name="cx_ps", bufs=1, space="PSUM"))
    tp_ps = ctx.enter_context(tc.tile_pool(name="tp_ps", bufs=1, space="PSUM"))

    # ---- constants -------------------------------------------------------
    ident = const.tile([P, P], bf16)
    masks.make_identity(nc, ident)

    W2bf = const.tile([P, DC, U], bf16)
    nc.gpsimd.dma_start(W2bf, W2.rearrange("(dc p) u -> p dc u", p=P))
    W1bf = const.tile([P, DC, U], bf16)
    nc.gpsimd.dma_start(W1bf, W1.rearrange("(dc p) u -> p dc u", p=P))
    Vbf = const.tile([P, UC, 1], bf16)
    nc.gpsimd.dma_start(Vbf, V.rearrange("(uc p) o -> p uc o", p=P))

    # ---- qprojT[u, b] = (query @ W1).T ----------------------------------
    q_sb = const.tile([BSH, D], f32)
    nc.sync.dma_start(q_sb, query)
    q_bf = const.tile([BSH, D], bf16)
    nc.vector.tensor_copy(q_bf, q_sb)
    qT = const.tile([P, DC, BSH], bf16)
    for dc in range(DC):
        tp = tp_ps.tile([P, BSH], bf16, tag="tp")
        nc.tensor.transpose(tp, q_bf[:, dc * P:(dc + 1) * P], ident[:BSH, :BSH])
        nc.vector.tensor_copy(qT[:, dc, :], tp)
    qpT = const.tile([P, UC, BSH], f32)
    for uc in range(UC):
        qp = tp_ps.tile([P, BSH], f32, tag="tp")
        for dc in range(DC):
            nc.tensor.matmul(qp, W1bf[:, dc, uc * P:(uc + 1) * P], qT[:, dc, :],
                             start=dc == 0, stop=dc == DC - 1)
        nc.vector.tensor_copy(qpT[:, uc, :], qp)

    # ---- per-batch pipeline ---------------------------------------------
    TPC = TT // TC  # t-tiles per t-chunk (4)

    def _batches():
      for b in range(BSH):
        vnat = npool.tile([P, TT, D], bf16, tag="vnat")
        logits = smax.tile([1, T], f32, tag="logits")
        for tc4 in range(TC):
            tsl = slice(tc4 * NF, (tc4 + 1) * NF)
            # stage this t-chunk: cast fp32->bf16 into DRAM, then
            # xbar-transpose into [d, t] layout + straight copy for context
            if sbuftr2:
                # cast-load straight into vnat (fp32->bf16, SWDGE, the only
                # HBM read of values), then one SBUF->SBUF xbar transpose
                # per t-tile: [128(t), 1024(d)] -> [128(d%128), 8(d/128),
                # 128(t)]; the matmul rhs uses a strided 3D AP to still
                # stream N=512.
                valT = vpool.tile([P, TPC, DC, P], bf16, tag="valT")
                if do_dma:
                    nc.gpsimd.dma_start(
                        vnat[:, tc4 * TPC:(tc4 + 1) * TPC, :],
                        values[b, tsl, :].rearrange("(tt p) d -> p tt d", p=P))
                    for tt in range(TPC):
                        nc.scalar.dma_start(valT[:, tt],
                                            vnat[:, tc4 * TPC + tt, :],
                                            transpose=True)
                rhs_of = lambda dc, vT=valT: vT[:, :, dc, :]
            elif sbuftr:
                valT = vpool.tile([P, DC, NF], bf16, tag="valT")
                if do_dma:
                    nc.gpsimd.dma_start(
                        vnat[:, tc4 * TPC:(tc4 + 1) * TPC, :],
                        values[b, tsl, :].rearrange("(tt p) d -> p tt d", p=P))
                    for tt in range(TPC):
                        for dc in range(DC):
                            nc.scalar.dma_start(
                                valT[:, dc, tt * P:(tt + 1) * P],
                                vnat[:, tc4 * TPC + tt, dc * P:(dc + 1) * P],
                                transpose=True)
                rhs_of = lambda dc, vT=valT: vT[:, dc, :]
            else:
                valT = vpool.tile([P, DC, NF], bf16, tag="valT")
                if do_dma:
                    valbf = dram.tile([NF, D], bf16, tag="valbf")
                    nc.gpsimd.dma_start(valbf, values[b, tsl, :])
                    nc.sync.dma_start_transpose(valT, valbf)
                    nc.sync.dma_start(
                        vnat[:, tc4 * TPC:(tc4 + 1) * TPC, :],
                        valbf.rearrange("(tt p) d -> p tt d", p=P))
                rhs_of = lambda dc, vT=valT: vT[:, dc, :]
            if not do_compute:
                junk = smax.tile([P, 8], bf16, tag="junk")
                nc.vector.tensor_copy(junk[:, 0:1], rhs_of(0)[:, 0, 0:1])
                nc.vector.tensor_copy(junk[:, 0:4],
                                      vnat[:, tc4 * TPC:(tc4 + 1) * TPC, 0])
                nc.vector.tensor_copy(logits[:, tc4:tc4 + 1], junk[0:1, 0:1])
                continue

            scoreT = spool.tile([P, UC, NF], bf16, tag="score")
            for uc in range(UC):
                ps = mm_ps.tile([P, NF], f32, tag="mm")
                for dc in range(DC):
                    nc.tensor.matmul(ps,
                                     W2bf[:, dc, uc * P:(uc + 1) * P],
                                     rhs_of(dc),
                                     start=dc == 0, stop=dc == DC - 1)
                nc.scalar.activation(scoreT[:, uc, :], ps, AF.Tanh,
                                     bias=qpT[:, uc, b:b + 1])
            # logits: 4 concurrent col-group M=1 matmuls (partials on
            # partitions 0/32/64/96), 2 accumulation rounds, DVE combine
            lps = lg_ps.tile([P, NF], f32, tag="lg")
            if coltile:
                for uc in range(UC):
                    j = uc % 4
                    nc.tensor.matmul(lps[32 * j:32 * j + 1, :],
                                     Vbf[:, uc, :], scoreT[:, uc, :],
                                     start=uc < 4, stop=uc >= 4,
                                     tile_position=(0, 32 * j))
                lgacc = smax.tile([1, NF], f32, tag="lgacc")
                nc.vector.tensor_copy(lgacc, lps[0:1, :])
                nc.vector.tensor_add(lgacc, lgacc, lps[32:33, :])
                nc.vector.tensor_add(lgacc, lgacc, lps[64:65, :])
                nc.vector.tensor_add(logits[:, tsl], lgacc, lps[96:97, :])
            else:
                for uc in range(UC):
                    nc.tensor.matmul(lps[0:1, :], Vbf[:, uc, :],
                                     scoreT[:, uc, :],
                                     start=uc == 0, stop=uc == UC - 1)
                nc.vector.tensor_copy(logits[:, tsl], lps[0:1, :])

        if not do_compute:
            nc.sync.dma_start(attn_out[b], logits)
            junk2 = smax.tile([1, D], f32, tag="junk2")
            nc.vector.tensor_copy(junk2[:, 0:4], logits[:, 0:4])
            nc.sync.dma_start(ctx_out[b], junk2)
            continue

        # softmax over the free axis on one partition
        mx = smax.tile([1, 1], f32, tag="mx")
        nc.vector.reduce_max(mx, logits, axis=mybir.AxisListType.X)
        negmx = smax.tile([1, 1], f32, tag="negmx")
        nc.vector.tensor_scalar_mul(negmx, mx, -1.0)
        probs = smax.tile([1, T], f32, tag="probs")
        sumexp = smax.tile([1, 1], f32, tag="sumexp")
        nc.scalar.activation(probs, logits, AF.Exp, bias=negmx,
                             accum_out=sumexp)
        rse = smax.tile([1, 1], f32, tag="rse")
        nc.vector.reciprocal(rse, sumexp)
        nc.vector.tensor_scalar_mul(probs, probs, rse)
        nc.sync.dma_start(attn_out[b], probs)

        # context: ctx[1, d] = sum_t attn[t] * values[t, d]
        attn16 = smax.tile([1, T], bf16, tag="attn16")
        nc.vector.tensor_copy(attn16, probs)
        # reshape [1, 2048] -> [16, 128] via SBUF->SBUF DMA, then one
        # PE transpose -> attnT [128(t%128), 16(t/128)]
        attnR = smax.tile([TT, P], bf16, tag="attnR")
        nc.sync.dma_start(attnR, attn16)
        tpA = tp_ps.tile([P, TT], bf16, tag="tp")
        nc.tensor.transpose(tpA, attnR, ident[:TT, :TT])
        attnT = smax.tile([P, TT], bf16, tag="attnT")
        nc.vector.tensor_copy(attnT, tpA)
        cxa = cx_ps.tile([P, NF], f32, tag="cx0")
        cxb = cx_ps.tile([P, NF], f32, tag="cx1")
        ctx_sb = smax.tile([1, D], f32, tag="ctx_sb")
        if coltile:
            for tt in range(TT):
                j = tt % 4
                nc.tensor.matmul(cxa[32 * j:32 * j + 1, :],
                                 attnT[:, tt:tt + 1], vnat[:, tt, 0:NF],
                                 start=tt < 4, stop=tt >= TT - 4,
                                 tile_position=(0, 32 * j))
                nc.tensor.matmul(cxb[32 * j:32 * j + 1, :],
                                 attnT[:, tt:tt + 1], vnat[:, tt, NF:D],
                                 start=tt < 4, stop=tt >= TT - 4,
                                 tile_position=(0, 32 * j))
            for cx, half in ((cxa, slice(0, NF)), (cxb, slice(NF, D))):
                hacc = smax.tile([1, NF], f32, tag="hacc")
                nc.vector.tensor_copy(hacc, cx[0:1, :])
                nc.vector.tensor_add(hacc, hacc, cx[32:33, :])
                nc.vector.tensor_add(hacc, hacc, cx[64:65, :])
                nc.vector.tensor_add(ctx_sb[:, half], hacc, cx[96:97, :])
        else:
            for tt in range(TT):
                nc.tensor.matmul(cxa[0:1, :], attnT[:, tt:tt + 1],
                                 vnat[:, tt, 0:NF],
                                 start=tt == 0, stop=tt == TT - 1)
                nc.tensor.matmul(cxb[0:1, :], attnT[:, tt:tt + 1],
                                 vnat[:, tt, NF:D],
                                 start=tt == 0, stop=tt == TT - 1)
            nc.vector.tensor_copy(ctx_sb[:, 0:NF], cxa[0:1, :])
            nc.vector.tensor_copy(ctx_sb[:, NF:D], cxb[0:1, :])
        nc.sync.dma_start(ctx_out[b], ctx_sb)

    if loop_n is not None and loop_n > 1:
        with tc.For_i(0, loop_n, 1) as _i:
            _batches()
    else:
        _batches()


def build():
    nc = bacc.Bacc("TRN2", target_bir_lowering=False, debug=False,
                   num_devices=NCORES)
    values = nc.dram_tensor("values", (BSH, T, D), f32,
                            kind="ExternalInput").ap()
    query = nc.dram_tensor("query", (BSH, D), f32, kind="ExternalInput").ap()
    W1 = nc.dram_tensor("W1", (D, U), f32, kind="ExternalInput").ap()
    W2 = nc.dram_tensor("W2", (D, U), f32, kind="ExternalInput").ap()
    V = nc.dram_tensor("V", (U, 1), f32, kind="ExternalInput").ap()
    ctx_out = nc.dram_tensor("ctx", (BSH, D), f32, kind="ExternalOutput").ap()
    attn_out = nc.dram_tensor("attn", (BSH, T), f32,
                              kind="ExternalOutput").ap()
    with tile.TileContext(nc) as tc, ExitStack() as ctx:
        _body(ctx, tc, values, query, W1, W2, V, ctx_out, attn_out)
    nc.compile()
    return nc


_NC = None


def _get_nc():
    global _NC
    if _NC is None:
        _NC = build()
    return _NC


def _run(in_maps, **kwargs):
    nc = _get_nc()
    return bass_utils.run_bass_kernel_spmd(nc, in_maps,
                                           core_ids=list(range(NCORES)),
                                           **kwargs)


def make_in_maps(query, values, W1, W2, V):
    query = np.ascontiguousarray(np.asarray(query, dtype=np.float32))
    values = np.ascontiguousarray(np.asarray(values, dtype=np.float32))
    W1 = np.ascontiguousarray(np.asarray(W1, dtype=np.float32))
    W2 = np.ascontiguousarray(np.asarray(W2, dtype=np.float32))
    V = np.ascontiguousarray(np.asarray(V, dtype=np.float32))
    in_maps = []
    for c in range(NCORES):
        sl = slice(c * BSH, (c + 1) * BSH)
        in_maps.append({
            "values": np.ascontiguousarray(values[sl]),
            "query": np.ascontiguousarray(query[sl]),
            "W1": W1, "W2": W2, "V": V,
        })
    return in_maps


def assemble(results):
    context = np.concatenate([results[c]["ctx"] for c in range(NCORES)], 0)
    attn = np.concatenate([results[c]["attn"] for c in range(NCORES)], 0)
    return context.astype(np.float32), attn[..., None].astype(np.float32)


def kernel(query, values, W1, W2, V):
    in_maps = make_in_maps(query, values, W1, W2, V)
    res = _run(in_maps)
    return assemble(res.results)


if __name__ == "__main__":
    rng = np.random.default_rng(0)
    scale = 1.0 / np.sqrt(D)
    inputs = dict(
        query=rng.standard_normal((B, D), dtype=np.float32),
        values=rng.standard_normal((B, T, D), dtype=np.float32),
        W1=rng.standard_normal((D, U), dtype=np.float32) * scale,
        W2=rng.standard_normal((D, U), dtype=np.float32) * scale,
        V=rng.standard_normal((U, 1), dtype=np.float32) / np.sqrt(U),
    )
    ctx_np, attn_np = kernel(**inputs)
    print(ctx_np.shape, attn_np.shape, ctx_np.dtype, attn_np.dtype)


# revision 29
# speedup vs baseline: 595.0813x; 1.2842x over previous
"""Bahdanau attention Trainium2 kernel.

Problem (per full input):
    query [32, 1024], values [32, 2048, 1024], W1 [1024, 1024],
    W2 [1024, 1024], V [1024, 1]
    q_proj = query @ W1                       # [B, U]
    v_proj = values @ W2                      # [B, T, U]
    score  = tanh(q_proj[:, None] + v_proj)   # [B, T, U]
    logits = score @ V                        # [B, T, 1]
    attn   = softmax(logits, axis=1)          # [B, T, 1]
    ctx    = sum_t attn * values              # [B, D]
    returns (ctx, attn)

Sharding: data-parallel over batch, 4 batches per core on 8 cores.

Per-core plan (u-major "transposed" layout so that the q_proj add is a
free per-partition ACT bias, the logits reduction is a PE matmul, and
the softmax runs along the free axis):
    - weights cast to bf16 on load (SWDGE cast DMA)
    - values: fp32 DRAM -> bf16 DRAM (cast DMA), then
        * xbar DMA transpose  -> valT [128(d), 8, 2048(t)]  (rhs of main mm)
        * straight load       -> vnat [128(t), 16, 1024(d)] (context mm)
    - main mm: vprojT[u, t] += W2bf[d, u].T @ valT[d, t] (8 k-chunks)
    - ACT: scoreT = tanh(vprojT + qprojT[u] bias) -> bf16
    - logits mm: lhsT = Vbf [u, 1] -> logits [1, t]
    - softmax on [1, 2048] row (DVE/ACT, exp accum_out gives sum)
    - context mm: lhsT = attnT [t, 1], rhs = vnat -> ctx [1, d]
"""
import numpy as np
from contextlib import ExitStack

import concourse.bass as bass
import concourse.bacc as bacc
import concourse.tile as tile
from concourse import mybir, masks
from concourse import bass_utils

B, T, D, U = 32, 2048, 1024, 1024
NCORES = 8
BSH = B // NCORES          # 4 batches per core
P = 128
DC = D // P                # 8 contraction chunks
UC = U // P                # 8 u tiles
NF = 512                   # matmul free dim (one PSUM bank of fp32)
TC = T // NF               # 4 t chunks
TT = T // P                # 16 t tiles
f32 = mybir.dt.float32
bf16 = mybir.dt.bfloat16
AF = mybir.ActivationFunctionType


ALL_PARTS = frozenset({"dma", "compute", "coltile", "wb"})


def _body(ctx, tc, values, query, W1, W2, V, ctx_out, attn_out, loop_n=None,
          parts=ALL_PARTS):
    nc = tc.nc
    do_dma = "dma" in parts
    do_compute = "compute" in parts
    coltile = "coltile" in parts
    sbuftr = "sbuftr" in parts
    sbuftr2 = "sbuftr2" in parts
    wb = "wb" in parts
    const = ctx.enter_context(tc.tile_pool(name="const", bufs=1))
    dram = ctx.enter_context(tc.tile_pool(name="dram", bufs=6, space="DRAM"))
    vpool = ctx.enter_context(tc.tile_pool(name="vpool", bufs=4))
    npool = ctx.enter_context(tc.tile_pool(name="npool", bufs=2))
    spool = ctx.enter_context(tc.tile_pool(name="spool", bufs=2))
    smax = ctx.enter_context(tc.tile_pool(name="smax", bufs=2))
    mm_ps = ctx.enter_context(tc.tile_pool(name="mm_ps", bufs=3, space="PSUM"))
    lg_ps = ctx.enter_context(tc.tile_pool(name="lg_ps", bufs=2, space="PSUM"))
    cx_ps = ctx.enter_context(tc.tile_pool(name="cx_ps", bufs=1, space="PSUM"))
    tp_ps = ctx.enter_context(tc.tile_pool(name="tp_ps", bufs=1, space="PSUM"))

    # ---- constants -------------------------------------------------------
    ident = const.tile([P, P], bf16)
    masks.make_identity(nc, ident)

    W2bf = const.tile([P, DC, U], bf16)
    nc.gpsimd.dma_start(W2bf, W2.rearrange("(dc p) u -> p dc u", p=P))
    W1bf = const.tile([P, DC, U], bf16)
    nc.gpsimd.dma_start(W1bf, W1.rearrange("(dc p) u -> p dc u", p=P))
    Vbf = const.tile([P, UC, 1], bf16)
    nc.gpsimd.dma_start(Vbf, V.rearrange("(uc p) o -> p uc o", p=P))

    # ---- qprojT[u, b] = (query @ W1).T ----------------------------------
    q_sb = const.tile([BSH, D], f32)
    nc.sync.dma_start(q_sb, query)
    q_bf = const.tile([BSH, D], bf16)
    nc.vector.tensor_copy(q_bf, q_sb)
    qT = const.tile([P, DC, BSH], bf16)
    for dc in range(DC):
        tp = tp_ps.tile([P, BSH], bf16, tag="tp")
        nc.tensor.transpose(tp, q_bf[:, dc * P:(dc + 1) * P], ident[:BSH, :BSH])
        nc.vector.tensor_copy(qT[:, dc, :], tp)
    qpT = const.tile([P, UC, BSH], f32)
    for uc in range(UC):
        qp = tp_ps.tile([P, BSH], f32, tag="tp")
        for dc in range(DC):
            nc.tensor.matmul(qp, W1bf[:, dc, uc * P:(uc + 1) * P], qT[:, dc, :],
                             start=dc == 0, stop=dc == DC - 1)
        nc.vector.tensor_copy(qpT[:, uc, :], qp)

    # ---- per-batch pipeline ---------------------------------------------
    TPC = TT // TC  # t-tiles per t-chunk (4)

    def _batches():
      for b in range(BSH):
        vnat = npool.tile([P, TT, D], bf16, tag="vnat")
        logits = smax.tile([1, T], f32, tag="logits")
        for tc4 in range(TC):
            tsl = slice(tc4 * NF, (tc4 + 1) * NF)
            # stage this t-chunk: cast fp32->bf16 into DRAM, then
            # xbar-transpose into [d, t] layout + straight copy for context
            if sbuftr2:
                # cast-load straight into vnat (fp32->bf16, SWDGE, the only
                # HBM read of values), then one SBUF->SBUF xbar transpose
                # per t-tile: [128(t), 1024(d)] -> [128(d%128), 8(d/128),
                # 128(t)]; the matmul rhs uses a strided 3D AP to still
                # stream N=512.
                valT = vpool.tile([P, TPC, DC, P], bf16, tag="valT")
                if do_dma:
                    nc.gpsimd.dma_start(
                        vnat[:, tc4 * TPC:(tc4 + 1) * TPC, :],
                        values[b, tsl, :].rearrange("(tt p) d -> p tt d", p=P))
                    for tt in range(TPC):
                        nc.scalar.dma_start(valT[:, tt],
                                            vnat[:, tc4 * TPC + tt, :],
                                            transpose=True)
                rhs_of = lambda dc, vT=valT: vT[:, :, dc, :]
            elif sbuftr:
                valT = vpool.tile([P, DC, NF], bf16, tag="valT")
                if do_dma:
                    nc.gpsimd.dma_start(
                        vnat[:, tc4 * TPC:(tc4 + 1) * TPC, :],
                        values[b, tsl, :].rearrange("(tt p) d -> p tt d", p=P))
                    for tt in range(TPC):
                        for dc in range(DC):
                            nc.scalar.dma_start(
                                valT[:, dc, tt * P:(tt + 1) * P],
                                vnat[:, tc4 * TPC + tt, dc * P:(dc + 1) * P],
                                transpose=True)
                rhs_of = lambda dc, vT=valT: vT[:, dc, :]
            elif wb:
                # cast fp32->bf16 straight into vnat (only HBM read of
                # values), write back bf16 to DRAM, big xbar transpose from
                # DRAM. 64 MiB of HBM traffic vs 80 for the plain staged
                # scheme.
                valT = vpool.tile([P, DC, NF], bf16, tag="valT")
                if do_dma:
                    vsl = vnat[:, tc4 * TPC:(tc4 + 1) * TPC, :]
                    nc.gpsimd.dma_start(
                        vsl,
                        values[b, tsl, :].rearrange("(tt p) d -> p tt d", p=P))
                    valbf = dram.tile([NF, D], bf16, tag="valbf")
                    nc.sync.dma_start(
                        valbf.rearrange("(tt p) d -> p tt d", p=P), vsl)
                    nc.scalar.dma_start_transpose(valT, valbf)
                rhs_of = lambda dc, vT=valT: vT[:, dc, :]
            else:
                valT = vpool.tile([P, DC, NF], bf16, tag="valT")
                if do_dma:
                    valbf = dram.tile([NF, D], bf16, tag="valbf")
                    nc.gpsimd.dma_start(valbf, values[b, tsl, :])
                    nc.sync.dma_start_transpose(valT, valbf)
                    nc.sync.dma_start(
                        vnat[:, tc4 * TPC:(tc4 + 1) * TPC, :],
                        valbf.rearrange("(tt p) d -> p tt d", p=P))
                rhs_of = lambda dc, vT=valT: vT[:, dc, :]
            if not do_compute:
                junk = smax.tile([P, 8], bf16, tag="junk")
                nc.vector.tensor_copy(junk[:, 0:1], rhs_of(0)[:, 0, 0:1])
                nc.vector.tensor_copy(junk[:, 0:4],
                                      vnat[:, tc4 * TPC:(tc4 + 1) * TPC, 0])
                nc.vector.tensor_copy(logits[:, tc4:tc4 + 1], junk[0:1, 0:1])
                continue

            scoreT = spool.tile([P, UC, NF], bf16, tag="score")
            for uc in range(UC):
                ps = mm_ps.tile([P, NF], f32, tag="mm")
                for dc in range(DC):
                    nc.tensor.matmul(ps,
                                     W2bf[:, dc, uc * P:(uc + 1) * P],
                                     rhs_of(dc),
                                     start=dc == 0, stop=dc == DC - 1)
                nc.scalar.activation(scoreT[:, uc, :], ps, AF.Tanh,
                                     bias=qpT[:, uc, b:b + 1])
            # logits: 4 concurrent col-group M=1 matmuls (partials on
            # partitions 0/32/64/96), 2 accumulation rounds, DVE combine
            lps = lg_ps.tile([P, NF], f32, tag="lg")
            if coltile:
                for uc in range(UC):
                    j = uc % 4
                    nc.tensor.matmul(lps[32 * j:32 * j + 1, :],
                                     Vbf[:, uc, :], scoreT[:, uc, :],
                                     start=uc < 4, stop=uc >= 4,
                                     tile_position=(0, 32 * j))
                lgacc = smax.tile([1, NF], f32, tag="lgacc")
                nc.vector.tensor_copy(lgacc, lps[0:1, :])
                nc.vector.tensor_add(lgacc, lgacc, lps[32:33, :])
                nc.vector.tensor_add(lgacc, lgacc, lps[64:65, :])
                nc.vector.tensor_add(logits[:, tsl], lgacc, lps[96:97, :])
            else:
                for uc in range(UC):
                    nc.tensor.matmul(lps[0:1, :], Vbf[:, uc, :],
                                     scoreT[:, uc, :],
                                     start=uc == 0, stop=uc == UC - 1)
                nc.vector.tensor_copy(logits[:, tsl], lps[0:1, :])

        if not do_compute:
            nc.sync.dma_start(attn_out[b], logits)
            junk2 = smax.tile([1, D], f32, tag="junk2")
            nc.vector.tensor_copy(junk2[:, 0:4], logits[:, 0:4])
            nc.sync.dma_start(ctx_out[b], junk2)
            continue

        # softmax over the free axis on one partition
        mx = smax.tile([1, 1], f32, tag="mx")
        nc.vector.reduce_max(mx, logits, axis=mybir.AxisListType.X)
        negmx = smax.tile([1, 1], f32, tag="negmx")
        nc.vector.tensor_scalar_mul(negmx, mx, -1.0)
        probs = smax.tile([1, T], f32, tag="probs")
        sumexp = smax.tile([1, 1], f32, tag="sumexp")
        nc.scalar.activation(probs, logits, AF.Exp, bias=negmx,
                             accum_out=sumexp)
        rse = smax.tile([1, 1], f32, tag="rse")
        nc.vector.reciprocal(rse, sumexp)
        nc.vector.tensor_scalar_mul(probs, probs, rse)
        nc.sync.dma_start(attn_out[b], probs)

        # context: ctx[1, d] = sum_t attn[t] * values[t, d]
        attn16 = smax.tile([1, T], bf16, tag="attn16")
        nc.vector.tensor_copy(attn16, probs)
        # reshape [1, 2048] -> [16, 128] via SBUF->SBUF DMA, then one
        # PE transpose -> attnT [128(t%128), 16(t/128)]
        attnR = smax.tile([TT, P], bf16, tag="attnR")
        nc.sync.dma_start(attnR, attn16)
        tpA = tp_ps.tile([P, TT], bf16, tag="tp")
        nc.tensor.transpose(tpA, attnR, ident[:TT, :TT])
        attnT = smax.tile([P, TT], bf16, tag="attnT")
        nc.vector.tensor_copy(attnT, tpA)
        cxa = cx_ps.tile([P, NF], f32, tag="cx0")
        cxb = cx_ps.tile([P, NF], f32, tag="cx1")
        ctx_sb = smax.tile([1, D], f32, tag="ctx_sb")
        if coltile:
            for tt in range(TT):
                j = tt % 4
                nc.tensor.matmul(cxa[32 * j:32 * j + 1, :],
                                 attnT[:, tt:tt + 1], vnat[:, tt, 0:NF],
                                 start=tt < 4, stop=tt >= TT - 4,
                                 tile_position=(0, 32 * j))
                nc.tensor.matmul(cxb[32 * j:32 * j + 1, :],
                                 attnT[:, tt:tt + 1], vnat[:, tt, NF:D],
                                 start=tt < 4, stop=tt >= TT - 4,
                                 tile_position=(0, 32 * j))
            for cx, half in ((cxa, slice(0, NF)), (cxb, slice(NF, D))):
                hacc = smax.tile([1, NF], f32, tag="hacc")
                nc.vector.tensor_copy(hacc, cx[0:1, :])
                nc.vector.tensor_add(hacc, hacc, cx[32:33, :])
                nc.vector.tensor_add(hacc, hacc, cx[64:65, :])
                nc.vector.tensor_add(ctx_sb[:, half], hacc, cx[96:97, :])
        else:
            for tt in range(TT):
                nc.tensor.matmul(cxa[0:1, :], attnT[:, tt:tt + 1],
                                 vnat[:, tt, 0:NF],
                                 start=tt == 0, stop=tt == TT - 1)
                nc.tensor.matmul(cxb[0:1, :], attnT[:, tt:tt + 1],
                                 vnat[:, tt, NF:D],
                                 start=tt == 0, stop=tt == TT - 1)
            nc.vector.tensor_copy(ctx_sb[:, 0:NF], cxa[0:1, :])
            nc.vector.tensor_copy(ctx_sb[:, NF:D], cxb[0:1, :])
        nc.sync.dma_start(ctx_out[b], ctx_sb)

    if loop_n is not None and loop_n > 1:
        with tc.For_i(0, loop_n, 1) as _i:
            _batches()
    else:
        _batches()


def build():
    nc = bacc.Bacc("TRN2", target_bir_lowering=False, debug=False,
                   num_devices=NCORES)
    values = nc.dram_tensor("values", (BSH, T, D), f32,
                            kind="ExternalInput").ap()
    query = nc.dram_tensor("query", (BSH, D), f32, kind="ExternalInput").ap()
    W1 = nc.dram_tensor("W1", (D, U), f32, kind="ExternalInput").ap()
    W2 = nc.dram_tensor("W2", (D, U), f32, kind="ExternalInput").ap()
    V = nc.dram_tensor("V", (U, 1), f32, kind="ExternalInput").ap()
    ctx_out = nc.dram_tensor("ctx", (BSH, D), f32, kind="ExternalOutput").ap()
    attn_out = nc.dram_tensor("attn", (BSH, T), f32,
                              kind="ExternalOutput").ap()
    with tile.TileContext(nc) as tc, ExitStack() as ctx:
        _body(ctx, tc, values, query, W1, W2, V, ctx_out, attn_out)
    nc.compile()
    return nc


_NC = None


def _get_nc():
    global _NC
    if _NC is None:
        _NC = build()
    return _NC


def _run(in_maps, **kwargs):
    nc = _get_nc()
    return bass_utils.run_bass_kernel_spmd(nc, in_maps,
                                           core_ids=list(range(NCORES)),
                                           **kwargs)


def make_in_maps(query, values, W1, W2, V):
    query = np.ascontiguousarray(np.asarray(query, dtype=np.float32))
    values = np.ascontiguousarray(np.asarray(values, dtype=np.float32))
    W1 = np.ascontiguousarray(np.asarray(W1, dtype=np.float32))
    W2 = np.ascontiguousarray(np.asarray(W2, dtype=np.float32))
    V = np.ascontiguousarray(np.asarray(V, dtype=np.float32))
    in_maps = []
    for c in range(NCORES):
        sl = slice(c * BSH, (c + 1) * BSH)
        in_maps.append({
            "values": np.ascontiguousarray(values[sl]),
            "query": np.ascontiguousarray(query[sl]),
            "W1": W1, "W2": W2, "V": V,
        })
    return in_maps


def assemble(results):
    context = np.concatenate([results[c]["ctx"] for c in range(NCORES)], 0)
    attn = np.concatenate([results[c]["attn"] for c in range(NCORES)], 0)
    return context.astype(np.float32), attn[..., None].astype(np.float32)


def kernel(query, values, W1, W2, V):
    in_maps = make_in_maps(query, values, W1, W2, V)
    res = _run(in_maps)
    return assemble(res.results)


if __name__ == "__main__":
    rng = np.random.default_rng(0)
    scale = 1.0 / np.sqrt(D)
    inputs = dict(
        query=rng.standard_normal((B, D), dtype=np.float32),
        values=rng.standard_normal((B, T, D), dtype=np.float32),
        W1=rng.standard_normal((D, U), dtype=np.float32) * scale,
        W2=rng.standard_normal((D, U), dtype=np.float32) * scale,
        V=rng.standard_normal((U, 1), dtype=np.float32) / np.sqrt(U),
    )
    ctx_np, attn_np = kernel(**inputs)
    print(ctx_np.shape, attn_np.shape, ctx_np.dtype, attn_np.dtype)
